# revision 32
# baseline (speedup 1.0000x reference)
"""Trainium2 Bass kernel for GCN-biased sparse attention (nn_Attention_37589553775245).

Reference computation (per batch b of 8, one NeuronCore each):
    qkv = x @ w_qkv; q,k,v per head (H=8, DH=64)
    attn = softmax(q k^T / sqrt(DH)) + A_hat        (A_hat = D^-1/2 (ceil(adj)+I) D^-1/2)
    out = (attn @ v) @ w_out + b_out

Sharding: pure batch-parallel across the 8 cores (B=8). A_hat is computed on
host (cheap) and replicated; weights replicated. No collectives.

v2 design (bf16 SBUF-resident data, fp32 PSUM accumulation):
  - All inputs are cast to bf16 on host: halves DMA and SBUF footprint, and
    every matmul runs at 1 cycle/row. With the smaller tiles EVERYTHING is
    SBUF-resident from t=0 (no deferred A_hat allocation), and the input DMAs
    are split across three rings (sync/vector/scalar) in gating order so the
    first projection matmul starts ~2us after kernel start.
  - q,k produced transposed (qT,kT [DH, N]); scores computed transposed
    (sT[j,i]) so the softmax denominator rides the attn@v matmul via an
    augmented V: vaug[n, h, 0:66] = [1 | v_h | 1]; even heads use cols 1:66
    (v|1 -> denom at out row 64), odd heads cols 0:65 (1|v -> denom at out
    row 63, v at rows 64:128). This makes the post-normalization DVE multiply
    lane-aligned with yE's row range for BOTH head parities - no partition
    shift, no DRAM bounce.
  - Softmax normalization: reciprocal of the ridden denominator row (DVE),
    partition_broadcast on the otherwise-idle Pool engine (no PE/PSUM/DMA
    cost), then one DVE multiply into yE. The broadcast+multiply for unit i
    is emitted inside unit i+1 so nothing stalls on the reciprocal.
  - Attention units are ordered i-chunk-outer (all 8 heads of chunk 0, then
    chunk 1) so chunk 0's merges + output projection overlap chunk 1's
    attention stream; remaining q/k tiles, A_hat@V units, yT merges and the
    out-projection are woven into per-unit filler slots at jb granularity to
    keep the PE fed while ACT chews the exp stream.
  - A_hat @ V reads v directly out of vaug (strided AP over the two heads of
    an f-tile) - no separate v tensor, no extra copies.
"""

import os
import sys

import numpy as np

for _p in ("/opt/trn_rl_repo", "/root/.axon_site/_ro/trn_rl_repo"):
    if _p not in sys.path and os.path.isdir(_p):
        sys.path.insert(0, _p)

import ml_dtypes  # noqa: E402

import concourse.bass as bass  # noqa: E402
import concourse.mybir as mybir  # noqa: E402
import concourse.tile as tile  # noqa: E402
from concourse import bacc  # noqa: E402
from concourse.bass_utils import run_bass_kernel_spmd  # noqa: E402

B, N, DIM, H, DH = 8, 1024, 512, 8, 64
F = H * DH          # 512, inner dim
NT = N // 128       # 8 n-tiles (also j-tiles)
DT = DIM // 128     # 4 dim-tiles
FT = F // 128       # 4 f-tiles
NC2 = N // 512      # 2 i-chunks of 512
SCALE = DH ** -0.5

F32 = mybir.dt.float32
BF = mybir.dt.bfloat16
BF_NP = ml_dtypes.bfloat16

_PROGRAM = None
_last_in_maps = None


def _build_program(reps=1, exp_batch=2, exps_bufs=6, small_bufs=3, bc_bufs=3,
                   sx_bufs=2, so_bufs=2, ss_bufs=2):
    nc = bacc.Bacc("TRN2", target_bir_lowering=False, debug=False, num_devices=8)

    xT_d = nc.dram_tensor("xT", [DIM, N], BF, kind="ExternalInput")
    wqkv_d = nc.dram_tensor("wqkv", [DIM, 3 * F], BF, kind="ExternalInput")
    ahatT_d = nc.dram_tensor("ahatT", [N, N], BF, kind="ExternalInput")
    wout_d = nc.dram_tensor("wout", [F, DIM], BF, kind="ExternalInput")
    bout_d = nc.dram_tensor("bout", [1, DIM], F32, kind="ExternalInput")
    out_d = nc.dram_tensor("out", [N, DIM], BF, kind="ExternalOutput")

    NJB = NT // exp_batch

    with tile.TileContext(nc) as tc:
        with (
            tc.tile_pool(name="big", bufs=1) as big,
            tc.tile_pool(name="ps_s", bufs=ss_bufs, space="PSUM") as ps_s,
            tc.tile_pool(name="ps_o", bufs=so_bufs, space="PSUM") as ps_o,
            tc.tile_pool(name="ps_x", bufs=sx_bufs, space="PSUM") as ps_x,
            tc.tile_pool(name="exps", bufs=exps_bufs) as exps,
            tc.tile_pool(name="small", bufs=small_bufs) as small,
            tc.tile_pool(name="bcp", bufs=bc_bufs) as bcp,
            tc.tile_pool(name="dscr", bufs=4, space="DRAM") as dscr,
            tc.tile_pool(name="outs", bufs=6) as outs,
        ):
          for _rep in range(reps):
            wout = big.tile([128, FT, DIM], BF)
            qkT = big.tile([128, 2 * FT, N], BF)     # tiles 0..3 q, 4..7 k
            vaug = big.tile([128, NT, H, DH + 1], BF)  # [v_h | 1] per head
            v_sb = big.tile([128, NT, F], BF)        # contiguous v for A_hat
            yTs = [big.tile([128, N], BF, name=f"yT{i}") for i in range(FT)]
            yEs = [big.tile([128, N], BF, name=f"yE{i}") for i in range(FT)]
            ahatT = big.tile([128, NT, N], BF)
            xT = big.tile([128, DT, N], BF)
            wqkv = big.tile([128, DT, 3 * F], BF)
            bout_bc = big.tile([128, DIM], F32)

            # PE warmup: dummy matmuls on a zeroed scratch tile keep the PE
            # busy from t~0 so the clock is fully ramped (2.4GHz needs 3us of
            # continuous execution) when the first gated matmul arrives.
            warm = big.tile([128, 512], BF, name="warm")
            ones1 = big.tile([128, 64], BF, name="ones1")
            nc.gpsimd.memset(ones1, 1.0)
            nc.vector.memset(warm, 0.0)
            for wi in range(20):
                pw = ps_o.tile([65, 256], F32, tag="po")
                nc.tensor.matmul(pw, warm[:, 0:65], warm[:, 0:256])

            # only the 64 ones-column cells need initializing (v copies fill
            # the rest); keep it off the DVE queue, which gates the first
            # attention unit via the qk/v PSUM->SBUF copies
            nc.gpsimd.memset(vaug[:, :, :, DH:DH + 1], 1.0)

            # ---- input loads, three rings, gating order ----------------
            def ld_w(lo, hi):
                nc.sync.dma_start(
                    out=wqkv[:, :, lo:hi],
                    in_=wqkv_d[:, lo:hi].rearrange("(t p) f -> p t f", p=128))

            nc.scalar.dma_start(
                out=xT[:, :, 0:512],
                in_=xT_d[:, 0:512].rearrange("(t p) n -> p t n", p=128))
            ld_w(0, 128)        # q ft0
            ld_w(512, 640)      # k ft0
            nc.scalar.dma_start(
                out=xT[:, :, 512:1024],
                in_=xT_d[:, 512:1024].rearrange("(t p) n -> p t n", p=128))
            ld_w(1024, 1280)    # v lo
            ld_w(1280, 1536)    # v hi
            ld_w(128, 256)      # q ft1
            ld_w(640, 768)      # k ft1
            ld_w(256, 512)      # q ft2/3
            ld_w(768, 1024)     # k ft2/3
            nc.sync.dma_start(
                out=wout, in_=wout_d[:, :].rearrange("(t p) n -> p t n", p=128))
            nc.sync.dma_start(out=bout_bc,
                              in_=bout_d[0:1, :].to_broadcast((128, DIM)))
            # A_hat chunks ride the SAME (sync) ring LAST: per-ring FIFO
            # guarantees they can't starve the gating loads above (DMA
            # dispatch doesn't wait for preceding engine work, so emitting
            # them later in the stream would not delay them)
            for q in range(4):
                nc.sync.dma_start(
                    out=ahatT[:, :, q * 256:(q + 1) * 256],
                    in_=ahatT_d[:, q * 256:(q + 1) * 256].rearrange(
                        "(t p) n -> p t n", p=128))

            # ---- work units --------------------------------------------
            def qk_chunk(ft, c):
                ps = ps_x.tile([128, 512], F32, tag="x")
                for dt_i in range(DT):
                    nc.tensor.matmul(
                        ps, wqkv[:, dt_i, ft * 128:(ft + 1) * 128],
                        xT[:, dt_i, c * 512:(c + 1) * 512],
                        start=(dt_i == 0), stop=(dt_i == DT - 1))
                nc.vector.tensor_copy(out=qkT[:, ft, c * 512:(c + 1) * 512],
                                      in_=ps)

            def v_unit(nt):
                ps = ps_x.tile([128, 512], F32, tag="x")
                for dt_i in range(DT):
                    nc.tensor.matmul(
                        ps, xT[:, dt_i, nt * 128:(nt + 1) * 128],
                        wqkv[:, dt_i, 2 * F:3 * F],
                        start=(dt_i == 0), stop=(dt_i == DT - 1))
                nc.vector.tensor_copy(
                    out=vaug[:, nt, :, 0:DH],
                    in_=ps.rearrange("p (h d) -> p h d", h=H))
                nc.vector.tensor_copy(out=v_sb[:, nt, :], in_=ps)

            def ahat_unit(ft, c):
                # (A_hat @ V)^T f-tile ft, i-chunk c; v read out of vaug
                ps = ps_x.tile([128, 512], F32, tag="x")
                for jt in range(NT):
                    nc.tensor.matmul(
                        ps, v_sb[:, jt, ft * 128:(ft + 1) * 128],
                        ahatT[:, jt, c * 512:(c + 1) * 512],
                        start=(jt == 0), stop=(jt == NT - 1))
                nc.vector.tensor_copy(out=yTs[ft][:, c * 512:(c + 1) * 512],
                                      in_=ps)

            def merge(ft, c):
                sl = slice(c * 512, (c + 1) * 512)
                nc.vector.tensor_add(yTs[ft][:, sl], yTs[ft][:, sl],
                                     yEs[ft][:, sl])

            def out_store(nt, ps):
                # the bias-add must run on DVE: the Pool engine has no PSUM
                # port (the BIR verifier rejects Pool reads of PSUM)
                ot = outs.tile([128, DIM], BF, tag="ot")
                nc.vector.tensor_add(ot, ps, bout_bc)
                eng = nc.sync if nt % 2 == 0 else nc.scalar
                eng.dma_start(out=out_d[nt * 128:(nt + 1) * 128, :], in_=ot)

            def outproj(nt):
                ps = ps_x.tile([128, 512], F32, tag="x")
                for ft in range(FT):
                    nc.tensor.matmul(
                        ps, yTs[ft][:, nt * 128:(nt + 1) * 128], wout[:, ft, :],
                        start=(ft == 0), stop=(ft == FT - 1))
                out_store(nt, ps)

            # split out-projection for the last four row-tiles: the ft0..2
            # partial accumulations run during the final attention unit (they
            # only need the earlier merges); after the last merge only one
            # 213ns ft3 matmul per tile remains on the critical tail.
            op_ps = {}

            def op_alloc():
                a = ps_s.tile([128, 2, 512], F32, tag="ps", name="opA")
                b = ps_s.tile([128, 2, 512], F32, tag="ps", name="opB")
                for i, nt in enumerate(range(4, NT)):
                    op_ps[nt] = (a if i < 2 else b)[:, i % 2, :]

            def op_partial(nt):
                # yT3 here is the A_hat-only part (merge(3,1) never runs);
                # the exp-attention part enters via yE3 in op_finish
                for ft in range(FT):
                    nc.tensor.matmul(
                        op_ps[nt], yTs[ft][:, nt * 128:(nt + 1) * 128],
                        wout[:, ft, :], start=(ft == 0), stop=False)

            def op_finish(nt):
                nc.tensor.matmul(
                    op_ps[nt], yEs[FT - 1][:, nt * 128:(nt + 1) * 128],
                    wout[:, FT - 1, :], start=False, stop=True)

            def attn_unit(h, c, tailB_prev, fillers, late=(), pe_bcast=False):
                even = (h % 2 == 0)
                hb = (h % 2) * 64
                ht = h // 2
                po = ps_o.tile([65, 512], F32, tag="po")
                ets = [None] * NJB
                fq = list(fillers)

                def sc(jb):
                    ps_sc = ps_s.tile([128, exp_batch, 512], F32, tag="ps")
                    for e in range(exp_batch):
                        jt = jb * exp_batch + e
                        nc.tensor.matmul(
                            ps_sc[:, e, :],
                            qkT[hb:hb + 64, FT + ht, jt * 128:(jt + 1) * 128],
                            qkT[hb:hb + 64, ht, c * 512:(c + 1) * 512])
                    et = exps.tile([128, exp_batch, 512], BF, tag="exp")
                    nc.scalar.activation(
                        out=et, in_=ps_sc,
                        func=mybir.ActivationFunctionType.Exp,
                        scale=float(SCALE))
                    ets[jb] = et

                def av(jb):
                    for e in range(exp_batch):
                        jt = jb * exp_batch + e
                        nc.tensor.matmul(po, vaug[:, jt, h, :],
                                         ets[jb][:, e, :],
                                         start=(jt == 0), stop=(jt == NT - 1))

                def fill():
                    if fq:
                        for f in fq.pop(0):
                            f()

                sc(0)
                sc(1)
                if tailB_prev is not None:
                    tailB_prev()     # Pool bcast + DVE mul, no PE work
                av(0)
                fill()
                sc(2)
                av(1)
                fill()
                sc(3)
                av(2)
                fill()
                av(3)
                while fq:            # leftover fillers
                    for f in fq.pop(0):
                        f()

                rt = small.tile([128, 512], BF, tag="rt")
                with nc.allow_low_precision(reason="bf16 softmax recip is ample"):
                    nc.vector.reciprocal(out=rt[64:65, :], in_=po[64:65, :])
                for lf in late:      # PE work overlapping the recip/tail chain
                    lf()

                def tailB():
                    # partition-broadcast of the reciprocal row via a DRAM
                    # bounce (SBUF sources can't broadcast across partitions;
                    # the gpsimd partition_broadcast ucode reads partition 0,
                    # not the AP's partition offset, so it can't be used here)
                    ysl = yEs[ht][hb:hb + 64, c * 512:(c + 1) * 512]
                    scr = dscr.tile([1, 512], BF, tag="scr")
                    nc.sync.dma_start(out=scr, in_=rt[64:65, :])
                    bc = bcp.tile([64, 512], BF, tag="bc")
                    nc.sync.dma_start(out=bc, in_=scr.to_broadcast((64, 512)))
                    if even:
                        nc.vector.tensor_mul(ysl, po[0:64, :], bc)
                    else:
                        # DVE lanes can't shift partitions; write at base 0
                        # and SWDGE-copy into the base-64 yE slice
                        prod = small.tile([64, 512], BF, tag="prod")
                        nc.vector.tensor_mul(prod, po[0:64, :], bc)
                        nc.gpsimd.dma_start(out=ysl, in_=prod)

                return tailB

            # ---- emission schedule -------------------------------------
            # minimal prologue: only what gates attention unit 0 (h=1, c=0):
            # q-ft0 chunk 0, k-ft0 both chunks, v tiles 0/1. The rest of v
            # and all remaining q/k chunks are woven into the unit stream at
            # jb granularity, just ahead of the av that consumes them.
            qk_chunk(0, 0)
            qk_chunk(4, 0)
            qk_chunk(4, 1)
            v_unit(0)
            v_unit(1)

            L = lambda f, *a: (lambda: f(*a))
            fillers_by_unit = [
                [[L(v_unit, 2), L(v_unit, 3)],
                 [L(v_unit, 4), L(v_unit, 5)],
                 [L(v_unit, 6), L(v_unit, 7)]],
                [[L(qk_chunk, 0, 1)],
                 [L(qk_chunk, 1, 0), L(qk_chunk, 5, 0)],
                 [L(qk_chunk, 5, 1)]],
                [[L(qk_chunk, 2, 0), L(qk_chunk, 6, 0)],
                 [L(qk_chunk, 6, 1)],
                 []],
                [[L(ahat_unit, 0, 0)], [L(merge, 0, 0)], [L(qk_chunk, 1, 1)]],
                [[L(qk_chunk, 3, 0), L(qk_chunk, 7, 0)], [L(qk_chunk, 7, 1)],
                 []],
                [[L(ahat_unit, 1, 0)], [L(merge, 1, 0)], [L(qk_chunk, 2, 1)]],
                [[L(ahat_unit, 2, 0)], [L(qk_chunk, 3, 1)], []],
                [[L(merge, 2, 0)], [], []],
                [[L(ahat_unit, 3, 0)], [L(merge, 3, 0)], []],
                [[L(outproj, 0)], [L(outproj, 1)], [L(outproj, 2)]],
                [[L(outproj, 3)], [L(ahat_unit, 0, 1)], []],
                [[L(merge, 0, 1)], [], []],
                [[L(ahat_unit, 1, 1)], [L(merge, 1, 1)], []],
                [[L(ahat_unit, 2, 1)], [], []],
                [[L(merge, 2, 1)], [], []],
                [[L(ahat_unit, 3, 1)], [], []],
            ]
            # odd head of each pair first so the pair's last tail (even) has
            # no SWDGE hop, and the final unit's tail is a plain DVE multiply
            units = [(h, c) for c in range(NC2) for h in (1, 0, 3, 2, 5, 4, 7, 6)]
            tailB_prev = None
            for ui, ((h, c), fillers) in enumerate(zip(units, fillers_by_unit)):
                last = ui == len(units) - 1
                late = ([op_alloc, L(op_partial, 4), L(op_partial, 5)]
                        if last else ())
                tailB_prev = attn_unit(h, c, tailB_prev, fillers, late=late,
                                       pe_bcast=last)
            tailB_prev()
            op_partial(6)
            op_partial(7)
            # all four yE3 matmuls first (PSUM deps are tile-coarse: a store
            # reading slice 0 would serialize the matmul writing slice 1)
            for nt in range(4, NT):
                op_finish(nt)
            for nt in range(4, NT):
                out_store(nt, op_ps[nt])

    nc.compile()
    return nc


def _get_program():
    global _PROGRAM
    if _PROGRAM is None:
        _PROGRAM = _build_program()
    return _PROGRAM


def kernel(x, adj, w_qkv, w_out, b_out):
    x = np.asarray(x, dtype=np.float32)
    adj = np.asarray(adj, dtype=np.float32)
    w_qkv = np.ascontiguousarray(np.asarray(w_qkv, dtype=np.float32))
    w_out = np.ascontiguousarray(np.asarray(w_out, dtype=np.float32))
    b_out = np.asarray(b_out, dtype=np.float32).reshape(1, DIM)

    # host-side: normalized adjacency bias, replicated (cheap: one 1024^2 pass)
    A = np.ceil(adj) + np.eye(N, dtype=np.float32)
    dinv = A.sum(axis=1) ** -0.5
    A_hat = (A * dinv[:, None]) * dinv[None, :]
    ahatT = np.ascontiguousarray(A_hat.T).astype(BF_NP)

    wqkv_bf = w_qkv.astype(BF_NP)
    wout_bf = w_out.astype(BF_NP)

    nc = _get_program()
    in_maps = []
    for b in range(B):
        in_maps.append({
            "xT": np.ascontiguousarray(x[b].T).astype(BF_NP),
            "wqkv": wqkv_bf,
            "ahatT": ahatT,
            "wout": wout_bf,
            "bout": b_out,
        })
    global _last_in_maps
    _last_in_maps = in_maps
    res = run_bass_kernel_spmd(nc, in_maps, list(range(B)))
    out = np.stack([np.asarray(res.results[b]["out"]) for b in range(B)], axis=0)
    return out.astype(np.float32)


if __name__ == "__main__":
    rng = np.random.default_rng(0)
    x = rng.standard_normal((B, N, DIM), dtype=np.float32)
    adj = (rng.random((N, N), dtype=np.float32) < 0.05).astype(np.float32) * 0.5
    w_qkv = rng.standard_normal((DIM, 3 * F), dtype=np.float32) * DIM ** -0.5
    w_out = rng.standard_normal((F, DIM), dtype=np.float32) * F ** -0.5
    b_out = np.zeros(DIM, dtype=np.float32)
    out = kernel(x=x, adj=adj, w_qkv=w_qkv, w_out=w_out, b_out=b_out)
    print("out", out.shape, out.dtype, np.abs(out).max())


# revision 37
# speedup vs baseline: 1.3524x; 1.3524x over previous
"""Trainium2 Bass kernel for GCN-biased sparse attention (nn_Attention_37589553775245).

Reference computation (per batch b of 8, one NeuronCore each):
    qkv = x @ w_qkv; q,k,v per head (H=8, DH=64)
    attn = softmax(q k^T / sqrt(DH)) + A_hat        (A_hat = D^-1/2 (ceil(adj)+I) D^-1/2)
    out = (attn @ v) @ w_out + b_out

Sharding: pure batch-parallel across the 8 cores (B=8). A_hat is computed on
host (cheap) and replicated; weights replicated. No collectives.

v2 design (bf16 SBUF-resident data, fp32 PSUM accumulation):
  - All inputs are cast to bf16 on host: halves DMA and SBUF footprint, and
    every matmul runs at 1 cycle/row. With the smaller tiles EVERYTHING is
    SBUF-resident from t=0 (no deferred A_hat allocation), and the input DMAs
    are split across three rings (sync/vector/scalar) in gating order so the
    first projection matmul starts ~2us after kernel start.
  - q,k produced transposed (qT,kT [DH, N]); scores computed transposed
    (sT[j,i]) so the softmax denominator rides the attn@v matmul via an
    augmented V: vaug[n, h, 0:66] = [1 | v_h | 1]; even heads use cols 1:66
    (v|1 -> denom at out row 64), odd heads cols 0:65 (1|v -> denom at out
    row 63, v at rows 64:128). This makes the post-normalization DVE multiply
    lane-aligned with yE's row range for BOTH head parities - no partition
    shift, no DRAM bounce.
  - Softmax normalization: reciprocal of the ridden denominator row (DVE),
    partition_broadcast on the otherwise-idle Pool engine (no PE/PSUM/DMA
    cost), then one DVE multiply into yE. The broadcast+multiply for unit i
    is emitted inside unit i+1 so nothing stalls on the reciprocal.
  - Attention units are ordered i-chunk-outer (all 8 heads of chunk 0, then
    chunk 1) so chunk 0's merges + output projection overlap chunk 1's
    attention stream; remaining q/k tiles, A_hat@V units, yT merges and the
    out-projection are woven into per-unit filler slots at jb granularity to
    keep the PE fed while ACT chews the exp stream.
  - A_hat @ V reads v directly out of vaug (strided AP over the two heads of
    an f-tile) - no separate v tensor, no extra copies.
"""

import os
import sys

import numpy as np

for _p in ("/opt/trn_rl_repo", "/root/.axon_site/_ro/trn_rl_repo"):
    if _p not in sys.path and os.path.isdir(_p):
        sys.path.insert(0, _p)

import ml_dtypes  # noqa: E402

import concourse.bass as bass  # noqa: E402
import concourse.mybir as mybir  # noqa: E402
import concourse.tile as tile  # noqa: E402
from concourse import bacc  # noqa: E402
from concourse.bass_utils import run_bass_kernel_spmd  # noqa: E402

B, N, DIM, H, DH = 8, 1024, 512, 8, 64
F = H * DH          # 512, inner dim
NT = N // 128       # 8 n-tiles (also j-tiles)
DT = DIM // 128     # 4 dim-tiles
FT = F // 128       # 4 f-tiles
NC2 = N // 512      # 2 i-chunks of 512
SCALE = DH ** -0.5

F32 = mybir.dt.float32
BF = mybir.dt.bfloat16
BF_NP = ml_dtypes.bfloat16

_PROGRAMS = {}
_last_in_maps = None


def _build_program(reps=1, exp_batch=2, bias_free=False, exps_bufs=10, small_bufs=3, bc_bufs=3,
                   sx_bufs=2, so_bufs=2, ss_bufs=2):
    nc = bacc.Bacc("TRN2", target_bir_lowering=False, debug=False, num_devices=8)

    xT_d = nc.dram_tensor("xT", [DIM, N], BF, kind="ExternalInput")
    wqkv_d = nc.dram_tensor("wqkv", [DIM, 3 * F], BF, kind="ExternalInput")
    ahatT_d = nc.dram_tensor("ahatT", [N, N], BF, kind="ExternalInput")
    wout_d = nc.dram_tensor("wout", [F, DIM], BF, kind="ExternalInput")
    bout_d = nc.dram_tensor("bout", [1, DIM], F32, kind="ExternalInput")
    out_d = nc.dram_tensor("out", [N, DIM], BF, kind="ExternalOutput")

    NJB = NT // exp_batch

    with tile.TileContext(nc) as tc:
        with (
            tc.tile_pool(name="big", bufs=1) as big,
            tc.tile_pool(name="ps_s", bufs=ss_bufs, space="PSUM") as ps_s,
            tc.tile_pool(name="ps_o", bufs=so_bufs, space="PSUM") as ps_o,
            tc.tile_pool(name="ps_x", bufs=sx_bufs, space="PSUM") as ps_x,
            tc.tile_pool(name="exps", bufs=exps_bufs) as exps,
            tc.tile_pool(name="small", bufs=small_bufs) as small,
            tc.tile_pool(name="bcp", bufs=bc_bufs) as bcp,
            tc.tile_pool(name="dscr", bufs=4, space="DRAM") as dscr,
            tc.tile_pool(name="outs", bufs=6) as outs,
        ):
          for _rep in range(reps):
            wout = big.tile([128, FT, DIM], BF)
            qkT = big.tile([128, 2 * FT, N], BF)     # tiles 0..3 q, 4..7 k
            vaug = big.tile([128, NT, H, DH + 1], BF)  # [v_h | 1] per head
            v_sb = big.tile([128, NT, F], BF)        # contiguous v for A_hat
            yTs = [big.tile([128, N], BF, name=f"yT{i}") for i in range(FT)]
            yEs = [big.tile([128, N], BF, name=f"yE{i}") for i in range(FT)]
            ahatT = big.tile([128, NT, N], BF)
            xT = big.tile([128, DT, N], BF)
            wqkv = big.tile([128, DT, 3 * F], BF)
            bout_bc = big.tile([128, DIM], F32)

            # PE warmup: dummy matmuls on a zeroed scratch tile keep the PE
            # busy from t~0 so the clock is fully ramped (2.4GHz needs 3us of
            # continuous execution) when the first gated matmul arrives.
            warm = big.tile([128, 512], BF, name="warm")
            ones1 = big.tile([128, 64], BF, name="ones1")
            nc.gpsimd.memset(ones1, 1.0)
            nc.vector.memset(warm, 0.0)
            for wi in range(20):
                pw = ps_o.tile([65, 256], F32, tag="po")
                nc.tensor.matmul(pw, warm[:, 0:65], warm[:, 0:256])

            # only the 64 ones-column cells need initializing (v copies fill
            # the rest); keep it off the DVE queue, which gates the first
            # attention unit via the qk/v PSUM->SBUF copies
            nc.gpsimd.memset(vaug[:, :, :, DH:DH + 1], 1.0)

            # dummy exp forces the ACT table load during the DMA phase (on
            # hardware the first activation otherwise pays ~2.7us for it)
            twarm = small.tile([128, 512], BF, tag="rt")
            nc.scalar.activation(out=twarm[0:1, 0:32], in_=warm[0:1, 0:32],
                                 func=mybir.ActivationFunctionType.Exp)

            # ---- input loads, three rings, gating order ----------------
            def ld_w(lo, hi):
                nc.sync.dma_start(
                    out=wqkv[:, :, lo:hi],
                    in_=wqkv_d[:, lo:hi].rearrange("(t p) f -> p t f", p=128))

            nc.scalar.dma_start(
                out=xT[:, :, 0:512],
                in_=xT_d[:, 0:512].rearrange("(t p) n -> p t n", p=128))
            ld_w(0, 128)        # q ft0
            ld_w(512, 640)      # k ft0
            nc.scalar.dma_start(
                out=xT[:, :, 512:1024],
                in_=xT_d[:, 512:1024].rearrange("(t p) n -> p t n", p=128))
            ld_w(1024, 1280)    # v lo
            ld_w(1280, 1536)    # v hi
            ld_w(128, 256)      # q ft1
            ld_w(640, 768)      # k ft1
            ld_w(256, 512)      # q ft2/3
            ld_w(768, 1024)     # k ft2/3
            nc.sync.dma_start(
                out=wout, in_=wout_d[:, :].rearrange("(t p) n -> p t n", p=128))
            nc.sync.dma_start(out=bout_bc,
                              in_=bout_d[0:1, :].to_broadcast((128, DIM)))
            # A_hat chunks ride the SAME (sync) ring LAST: per-ring FIFO
            # guarantees they can't starve the gating loads above (DMA
            # dispatch doesn't wait for preceding engine work, so emitting
            # them later in the stream would not delay them)
            for q in range(4):
                nc.sync.dma_start(
                    out=ahatT[:, :, q * 256:(q + 1) * 256],
                    in_=ahatT_d[:, q * 256:(q + 1) * 256].rearrange(
                        "(t p) n -> p t n", p=128))

            # ---- work units --------------------------------------------
            def qk_chunk(ft, c):
                ps = ps_x.tile([128, 512], F32, tag="x")
                for dt_i in range(DT):
                    nc.tensor.matmul(
                        ps, wqkv[:, dt_i, ft * 128:(ft + 1) * 128],
                        xT[:, dt_i, c * 512:(c + 1) * 512],
                        start=(dt_i == 0), stop=(dt_i == DT - 1))
                nc.vector.tensor_copy(out=qkT[:, ft, c * 512:(c + 1) * 512],
                                      in_=ps)

            def v_unit(nt):
                ps = ps_x.tile([128, 512], F32, tag="x")
                for dt_i in range(DT):
                    nc.tensor.matmul(
                        ps, xT[:, dt_i, nt * 128:(nt + 1) * 128],
                        wqkv[:, dt_i, 2 * F:3 * F],
                        start=(dt_i == 0), stop=(dt_i == DT - 1))
                nc.vector.tensor_copy(
                    out=vaug[:, nt, :, 0:DH],
                    in_=ps.rearrange("p (h d) -> p h d", h=H))
                nc.vector.tensor_copy(out=v_sb[:, nt, :], in_=ps)

            def ahat_unit(ft, c):
                # (A_hat @ V)^T f-tile ft, i-chunk c; v read out of vaug
                ps = ps_x.tile([128, 512], F32, tag="x")
                for jt in range(NT):
                    nc.tensor.matmul(
                        ps, v_sb[:, jt, ft * 128:(ft + 1) * 128],
                        ahatT[:, jt, c * 512:(c + 1) * 512],
                        start=(jt == 0), stop=(jt == NT - 1))
                nc.vector.tensor_copy(out=yTs[ft][:, c * 512:(c + 1) * 512],
                                      in_=ps)

            def merge(ft, c):
                sl = slice(c * 512, (c + 1) * 512)
                nc.vector.tensor_add(yTs[ft][:, sl], yTs[ft][:, sl],
                                     yEs[ft][:, sl])

            def out_store(nt, ps, act_copy=False):
                # the bias-add must run on DVE: the Pool engine has no PSUM
                # port (the BIR verifier rejects Pool reads of PSUM). In the
                # bias-free specialization the closing stores alternate
                # DVE adds with ACT copies so the two chains run in parallel.
                ot = outs.tile([128, DIM], BF, tag="ot")
                if act_copy and bias_free:
                    nc.scalar.copy(out=ot, in_=ps)
                else:
                    nc.vector.tensor_add(ot, ps, bout_bc)
                eng = nc.sync if nt % 2 == 0 else nc.scalar
                eng.dma_start(out=out_d[nt * 128:(nt + 1) * 128, :], in_=ot)

            def outproj(nt):
                ps = ps_x.tile([128, 512], F32, tag="x")
                for ft in range(FT):
                    nc.tensor.matmul(
                        ps, yTs[ft][:, nt * 128:(nt + 1) * 128], wout[:, ft, :],
                        start=(ft == 0), stop=(ft == FT - 1))
                out_store(nt, ps)

            # split out-projection for the last four row-tiles: the ft0..2
            # partial accumulations run during the final attention unit (they
            # only need the earlier merges); after the last merge only one
            # 213ns ft3 matmul per tile remains on the critical tail.
            op_ps = {}

            def op_alloc():
                a = ps_s.tile([128, 2, 512], F32, tag="ps", name="opA")
                b = ps_s.tile([128, 2, 512], F32, tag="ps", name="opB")
                for i, nt in enumerate(range(4, NT)):
                    op_ps[nt] = (a if i < 2 else b)[:, i % 2, :]

            def op_partial(nt):
                # yT3 here is the A_hat-only part (merge(3,1) never runs);
                # the exp-attention part enters via yE3 in op_finish
                for ft in range(FT):
                    nc.tensor.matmul(
                        op_ps[nt], yTs[ft][:, nt * 128:(nt + 1) * 128],
                        wout[:, ft, :], start=(ft == 0), stop=False)

            def op_finish(nt):
                nc.tensor.matmul(
                    op_ps[nt], yEs[FT - 1][:, nt * 128:(nt + 1) * 128],
                    wout[:, FT - 1, :], start=False, stop=True)

            def attn_unit(h, c, tailB_prev, fillers, late=(), pe_bcast=False):
                even = (h % 2 == 0)
                hb = (h % 2) * 64
                ht = h // 2
                po = ps_o.tile([65, 512], F32, tag="po")
                ets = [None] * NJB
                fq = list(fillers)

                def sc(jb):
                    ps_sc = ps_s.tile([128, exp_batch, 512], F32, tag="ps")
                    for e in range(exp_batch):
                        jt = jb * exp_batch + e
                        nc.tensor.matmul(
                            ps_sc[:, e, :],
                            qkT[hb:hb + 64, FT + ht, jt * 128:(jt + 1) * 128],
                            qkT[hb:hb + 64, ht, c * 512:(c + 1) * 512])
                    et = exps.tile([128, exp_batch, 512], BF, tag="exp")
                    nc.scalar.activation(
                        out=et, in_=ps_sc,
                        func=mybir.ActivationFunctionType.Exp,
                        scale=float(SCALE))
                    ets[jb] = et

                def av(jb):
                    for e in range(exp_batch):
                        jt = jb * exp_batch + e
                        nc.tensor.matmul(po, vaug[:, jt, h, :],
                                         ets[jb][:, e, :],
                                         start=(jt == 0), stop=(jt == NT - 1))

                def fill():
                    if fq:
                        for f in fq.pop(0):
                            f()

                sc(0)
                sc(1)
                if tailB_prev is not None:
                    tailB_prev()     # Pool bcast + DVE mul, no PE work
                av(0)
                fill()
                sc(2)
                av(1)
                fill()
                sc(3)
                av(2)
                fill()
                av(3)
                while fq:            # leftover fillers
                    for f in fq.pop(0):
                        f()

                rt = small.tile([128, 512], BF, tag="rt")
                with nc.allow_low_precision(reason="bf16 softmax recip is ample"):
                    nc.vector.reciprocal(out=rt[64:65, :], in_=po[64:65, :])
                for lf in late:      # PE work overlapping the recip/tail chain
                    lf()

                def tailB():
                    # partition-broadcast of the reciprocal row. Normally via
                    # a DRAM bounce (SBUF sources can't broadcast across
                    # partitions; the gpsimd partition_broadcast ucode reads
                    # partition 0, not the AP's partition offset). The final
                    # unit instead uses a K=1 PE matmul (ones x recip row)
                    # plus an ACT copy to SBUF - both engines are idle there
                    # and it is ~0.5us shorter on the closing serial chain.
                    ysl = yEs[ht][hb:hb + 64, c * 512:(c + 1) * 512]
                    bc = bcp.tile([64, 512], BF, tag="bc")
                    if pe_bcast:
                        bp = ps_x.tile([64, 512], F32, tag="x")
                        nc.tensor.matmul(bp, ones1[64:65, :], rt[64:65, :])
                        nc.scalar.copy(out=bc, in_=bp)
                    else:
                        scr = dscr.tile([1, 512], BF, tag="scr")
                        nc.sync.dma_start(out=scr, in_=rt[64:65, :])
                        nc.sync.dma_start(out=bc,
                                          in_=scr.to_broadcast((64, 512)))
                    if even:
                        nc.vector.tensor_mul(ysl, po[0:64, :], bc)
                    else:
                        # DVE lanes can't shift partitions; write at base 0
                        # and SWDGE-copy into the base-64 yE slice
                        prod = small.tile([64, 512], BF, tag="prod")
                        nc.vector.tensor_mul(prod, po[0:64, :], bc)
                        nc.gpsimd.dma_start(out=ysl, in_=prod)

                return tailB

            # ---- emission schedule -------------------------------------
            # minimal prologue: only what gates attention unit 0 (h=1, c=0):
            # q-ft0 chunk 0, k-ft0 both chunks, v tiles 0/1. The rest of v
            # and all remaining q/k chunks are woven into the unit stream at
            # jb granularity, just ahead of the av that consumes them.
            qk_chunk(0, 0)
            qk_chunk(4, 0)
            qk_chunk(4, 1)
            v_unit(0)
            v_unit(1)

            L = lambda f, *a: (lambda: f(*a))
            fillers_by_unit = [
                [[L(v_unit, 2), L(v_unit, 3)],
                 [L(v_unit, 4), L(v_unit, 5)],
                 [L(v_unit, 6), L(v_unit, 7)]],
                [[L(qk_chunk, 0, 1)],
                 [L(qk_chunk, 1, 0), L(qk_chunk, 5, 0)],
                 [L(qk_chunk, 5, 1)]],
                [[L(qk_chunk, 2, 0), L(qk_chunk, 6, 0)],
                 [L(qk_chunk, 6, 1)],
                 []],
                [[L(ahat_unit, 0, 0)], [L(merge, 0, 0)], [L(qk_chunk, 1, 1)]],
                [[L(qk_chunk, 3, 0), L(qk_chunk, 7, 0)], [L(qk_chunk, 7, 1)],
                 []],
                [[L(ahat_unit, 1, 0)], [L(merge, 1, 0)], [L(qk_chunk, 2, 1)]],
                [[L(ahat_unit, 2, 0)], [L(qk_chunk, 3, 1)], []],
                [[L(merge, 2, 0)], [], []],
                [[L(ahat_unit, 3, 0)], [L(merge, 3, 0)], []],
                [[L(outproj, 0)], [L(outproj, 1)], [L(outproj, 2)]],
                [[L(outproj, 3)], [L(ahat_unit, 0, 1)], []],
                [[L(merge, 0, 1)], [], []],
                [[L(ahat_unit, 1, 1)], [L(merge, 1, 1)], []],
                [[L(ahat_unit, 2, 1)], [], []],
                [[L(merge, 2, 1)], [], []],
                [[L(ahat_unit, 3, 1)], [], []],
            ]
            # odd head of each pair first so the pair's last tail (even) has
            # no SWDGE hop, and the final unit's tail is a plain DVE multiply
            units = [(h, c) for c in range(NC2) for h in (1, 0, 3, 2, 5, 4, 7, 6)]
            tailB_prev = None
            for ui, ((h, c), fillers) in enumerate(zip(units, fillers_by_unit)):
                last = ui == len(units) - 1
                late = ([op_alloc, L(op_partial, 4), L(op_partial, 5)]
                        if last else ())
                tailB_prev = attn_unit(h, c, tailB_prev, fillers, late=late,
                                       pe_bcast=last)
            tailB_prev()
            op_partial(6)
            op_partial(7)
            # all four yE3 matmuls first (PSUM deps are tile-coarse: a store
            # reading slice 0 would serialize the matmul writing slice 1)
            for nt in range(4, NT):
                op_finish(nt)
            for nt in range(4, NT):
                out_store(nt, op_ps[nt])

    nc.compile()
    return nc


def _get_program(bias_free=False):
    if bias_free not in _PROGRAMS:
        _PROGRAMS[bias_free] = _build_program(bias_free=bias_free)
    return _PROGRAMS[bias_free]


def kernel(x, adj, w_qkv, w_out, b_out):
    x = np.asarray(x, dtype=np.float32)
    adj = np.asarray(adj, dtype=np.float32)
    w_qkv = np.ascontiguousarray(np.asarray(w_qkv, dtype=np.float32))
    w_out = np.ascontiguousarray(np.asarray(w_out, dtype=np.float32))
    b_out = np.asarray(b_out, dtype=np.float32).reshape(1, DIM)

    # host-side: normalized adjacency bias, replicated (cheap: one 1024^2 pass)
    A = np.ceil(adj) + np.eye(N, dtype=np.float32)
    dinv = A.sum(axis=1) ** -0.5
    A_hat = (A * dinv[:, None]) * dinv[None, :]
    ahatT = np.ascontiguousarray(A_hat.T).astype(BF_NP)

    wqkv_bf = w_qkv.astype(BF_NP)
    wout_bf = w_out.astype(BF_NP)

    nc = _get_program(bias_free=not np.any(b_out))
    in_maps = []
    for b in range(B):
        in_maps.append({
            "xT": np.ascontiguousarray(x[b].T).astype(BF_NP),
            "wqkv": wqkv_bf,
            "ahatT": ahatT,
            "wout": wout_bf,
            "bout": b_out,
        })
    global _last_in_maps
    _last_in_maps = in_maps
    res = run_bass_kernel_spmd(nc, in_maps, list(range(B)))
    out = np.stack([np.asarray(res.results[b]["out"]) for b in range(B)], axis=0)
    return out.astype(np.float32)


if __name__ == "__main__":
    rng = np.random.default_rng(0)
    x = rng.standard_normal((B, N, DIM), dtype=np.float32)
    adj = (rng.random((N, N), dtype=np.float32) < 0.05).astype(np.float32) * 0.5
    w_qkv = rng.standard_normal((DIM, 3 * F), dtype=np.float32) * DIM ** -0.5
    w_out = rng.standard_normal((F, DIM), dtype=np.float32) * F ** -0.5
    b_out = np.zeros(DIM, dtype=np.float32)
    out = kernel(x=x, adj=adj, w_qkv=w_qkv, w_out=w_out, b_out=b_out)
    print("out", out.shape, out.dtype, np.abs(out).max())
